# revision 6
# baseline (speedup 1.0000x reference)
"""Bass/Trainium2 kernel for nn_LSTMModel (B=128, T=512, D=256, H=512).

Sharding: data-parallel over batch across 8 NeuronCores (16 rows each),
weights replicated and SBUF-resident in bf16 transposed (lhsT) layout.

Scan layout: weight-stationary matmuls, gates on PSUM partitions.
Gate chunk m (0..15) covers gates [m*128, (m+1)*128); gate g = m*128 + p.
Gate type regions: i = m 0..3, f = 4..7, g = 8..11, o = 12..15.
h/c state layout: [128 part, (j, b)] with h-index = j*128 + p, so h state
slices [:, j*16:(j+1)*16] are directly the K-chunk rhs for the next matmul.
"""

import numpy as np

import concourse.bass as bass
import concourse.tile as tile
import concourse.mybir as mybir
from concourse import bacc
from concourse.bass import ds
from concourse.bass_utils import run_bass_kernel_spmd
from concourse.masks import make_identity

F32 = mybir.dt.float32
BF16 = mybir.dt.bfloat16
AF = mybir.ActivationFunctionType

B, T, D, H = 128, 512, 256, 512
NCORES = 8
BL = B // NCORES            # 16 batch rows per core
G = 4 * H                   # 2048 gates
MCH = G // 128              # 16 gate chunks
DKC = D // 128              # 2
HKC = H // 128              # 4

# L0 psum split: A holds f+i regions, Bt holds g, Ct holds o.
# slot list: (tile_key, offset, mc) in MM-emission order.
L0_SLOTS = (
    [("A", 16 * k, 4 + k) for k in range(4)]        # f
    + [("A", 64 + 16 * k, k) for k in range(4)]     # i
    + [("B", 16 * k, 8 + k) for k in range(4)]      # g
    + [("C", 16 * k, 12 + k) for k in range(4)]     # o
)


def build_nc(t_steps=T, unroll=8):
    assert t_steps % 8 == 0 and t_steps % unroll == 0
    ntot = t_steps * BL                   # total (t, b) columns
    nch = 512 if ntot % 512 == 0 else ntot  # xp matmul N chunk

    nc = bacc.Bacc("TRN2", target_bir_lowering=False)

    x = nc.dram_tensor("x", [BL, t_steps, D], F32, kind="ExternalInput")
    proj_w = nc.dram_tensor("proj_w", [D, D], F32, kind="ExternalInput")
    proj_b = nc.dram_tensor("proj_b", [D], F32, kind="ExternalInput")
    wx0 = nc.dram_tensor("wx0", [G, D], F32, kind="ExternalInput")
    bx0 = nc.dram_tensor("bx0", [G], F32, kind="ExternalInput")
    wh0 = nc.dram_tensor("wh0", [G, H], F32, kind="ExternalInput")
    bh0 = nc.dram_tensor("bh0", [G], F32, kind="ExternalInput")
    wx1 = nc.dram_tensor("wx1", [G, H], F32, kind="ExternalInput")
    bx1 = nc.dram_tensor("bx1", [G], F32, kind="ExternalInput")
    wh1 = nc.dram_tensor("wh1", [G, H], F32, kind="ExternalInput")
    bh1 = nc.dram_tensor("bh1", [G], F32, kind="ExternalInput")
    fc1_w = nc.dram_tensor("fc1_w", [32, H], F32, kind="ExternalInput")
    fc1_b = nc.dram_tensor("fc1_b", [32], F32, kind="ExternalInput")
    fc2_w = nc.dram_tensor("fc2_w", [1, 32], F32, kind="ExternalInput")
    fc2_b = nc.dram_tensor("fc2_b", [1], F32, kind="ExternalInput")
    out_d = nc.dram_tensor("out", [BL, 1], F32, kind="ExternalOutput")

    tens = dict(locals())
    with tile.TileContext(nc) as tc:
        with tc.tile_pool(name="res", bufs=1) as res, \
             tc.tile_pool(name="stg", bufs=3) as stg, \
             tc.tile_pool(name="scn", bufs=3) as scn, \
             tc.tile_pool(name="psum", bufs=2, space="PSUM") as psum:
            _build_body(nc, tc, res, stg, scn, psum, tens, t_steps,
                        unroll, ntot, nch)
    nc.compile()
    return nc


def _build_body(nc, tc, res, stg, scn, psum, tens, t_steps, unroll, ntot,
                nch):
    x, out_d = tens["x"], tens["out_d"]

    ident = res.tile([128, 128], F32, tag="ident")
    make_identity(nc, ident[:, :])

    # ---- resident transposed weights (bf16) ----
    w0T = res.tile([128, 6 * G], BF16, tag="w0T")    # kc 0..1 wx0, 2..5 wh0
    w1T = res.tile([128, 8 * G], BF16, tag="w1T")    # kc 0..3 wx1, 4..7 wh1
    for w_d, kcs, dst, kbase in ((tens["wx0"], DKC, w0T, 0),
                                 (tens["wh0"], HKC, w0T, DKC),
                                 (tens["wx1"], HKC, w1T, 0),
                                 (tens["wh1"], HKC, w1T, HKC)):
        cdim = w_d.shape[1]
        for gc in range(MCH):
            st = stg.tile([128, 512], F32, tag="wstage")
            nc.sync.dma_start(out=st[:, 0:cdim],
                              in_=w_d[gc * 128:(gc + 1) * 128, :])
            for kc in range(kcs):
                pt = psum.tile([128, 512], F32, tag="big")
                nc.tensor.transpose(pt[:, 0:128],
                                    st[:, kc * 128:(kc + 1) * 128],
                                    ident[:, :])
                o = ((kbase + kc) * MCH + gc) * 128
                nc.vector.tensor_copy(dst[:, o:o + 128], pt[:, 0:128])

    # proj_w.T (fp32, stays fp32 for the xp matmul)
    projT = res.tile([128, 2 * D], F32, tag="projT")
    for gc in range(DKC):
        st = stg.tile([128, 512], F32, tag="wstage")
        nc.sync.dma_start(out=st[:, 0:D],
                          in_=tens["proj_w"][gc * 128:(gc + 1) * 128, :])
        for kc in range(DKC):
            pt = psum.tile([128, 512], F32, tag="big")
            nc.tensor.transpose(pt[:, 0:128],
                                st[:, kc * 128:(kc + 1) * 128], ident[:, :])
            nc.vector.tensor_copy(projT[:, (kc * 2 + gc) * 128:
                                        (kc * 2 + gc) * 128 + 128],
                                  pt[:, 0:128])

    # fc1_w [32, 512] -> fc1T [128, 4*32] bf16 ; fc2_w [1,32] -> [32,1] bf16
    fc1T = res.tile([128, HKC * 32], BF16, tag="fc1T")
    st = stg.tile([128, 512], F32, tag="wstage")
    nc.sync.dma_start(out=st[0:32, :], in_=tens["fc1_w"][:, :])
    for kc in range(HKC):
        pt = psum.tile([128, 512], F32, tag="big")
        nc.tensor.transpose(pt[:, 0:32], st[0:32, kc * 128:(kc + 1) * 128],
                            ident[0:32, 0:32])
        nc.vector.tensor_copy(fc1T[:, kc * 32:(kc + 1) * 32], pt[:, 0:32])
    fc2T_f = res.tile([32, 1], F32, tag="fc2T_f")
    nc.sync.dma_start(out=fc2T_f[:, :],
                      in_=tens["fc2_w"][0:1, :].rearrange("o k -> k o"))
    fc2T = res.tile([32, 1], BF16, tag="fc2T")
    nc.vector.tensor_copy(fc2T[:, :], fc2T_f[:, :])
    fc1b = res.tile([32, 1], F32, tag="fc1b")
    nc.sync.dma_start(out=fc1b[:, :],
                      in_=tens["fc1_b"][:].rearrange("(k o) -> k o", o=1))
    fc2b = res.tile([1, 1], F32, tag="fc2b")
    nc.sync.dma_start(out=fc2b[:, :],
                      in_=tens["fc2_b"][:].rearrange("(k o) -> k o", o=1))

    # ---- gate biases: bsum[p, m] = (bx+bh)[m*128+p]; then broadcast ----
    def bias_cols(bsum, cols_dst, msel):
        # cols_dst[:, k*16:(k+1)*16] = bsum[:, msel[k]] broadcast over b
        for k, m in enumerate(msel):
            nc.vector.tensor_copy(
                cols_dst[:, k * 16:(k + 1) * 16],
                bsum[:, m:m + 1].to_broadcast([128, 16]))

    bsums = []
    for ba, bb in ((tens["bx0"], tens["bh0"]), (tens["bx1"], tens["bh1"])):
        parts = []
        for src in (ba, bb):
            st = stg.tile([16, 128], F32, tag="bstage")
            nc.sync.dma_start(out=st[:, :],
                              in_=src[:].rearrange("(m p) -> m p", p=128))
            pt = psum.tile([128, 512], F32, tag="big")
            nc.tensor.transpose(pt[:, 0:16], st[:, :], ident[0:16, 0:16])
            sb = stg.tile([128, 16], F32, tag="btp")
            nc.vector.tensor_copy(sb[:, :], pt[:, 0:16])
            parts.append(sb)
        tot = res.tile([128, 16], F32, tag=f"bsum{len(bsums)}")
        nc.vector.tensor_add(tot[:, :], parts[0][:, :], parts[1][:, :])
        bsums.append(tot)
    biasA = res.tile([128, 128], F32, tag="biasA")   # f then i
    biasB = res.tile([128, 64], F32, tag="biasB")    # g
    biasC = res.tile([128, 64], F32, tag="biasC")    # o
    bias_cols(bsums[0], biasA[:, 0:64], [4, 5, 6, 7])
    bias_cols(bsums[0], biasA[:, 64:128], [0, 1, 2, 3])
    bias_cols(bsums[0], biasB[:, :], [8, 9, 10, 11])
    bias_cols(bsums[0], biasC[:, :], [12, 13, 14, 15])
    bias1 = res.tile([128, 256], F32, tag="bias1")   # natural m order
    bias_cols(bsums[1], bias1[:, :], list(range(16)))

    # ---- x -> xT (fp32, PE transpose), column order n = t*16 + b ----
    xT = res.tile([128, DKC * ntot], F32, tag="xT")
    for rc in range(t_steps // 8):
        st = stg.tile([128, 256], F32, tag="xstage")
        for tt in range(8):
            nc.sync.dma_start(
                out=st[tt * 16:(tt + 1) * 16, :].rearrange(
                    "p (o d) -> p o d", o=1),
                in_=x[:, rc * 8 + tt:rc * 8 + tt + 1, :])
        for kc in range(DKC):
            pt = psum.tile([128, 512], F32, tag="big")
            nc.tensor.transpose(pt[:, 0:128],
                                st[:, kc * 128:(kc + 1) * 128], ident[:, :])
            nc.vector.tensor_copy(xT[:, kc * ntot + rc * 128:
                                     kc * ntot + rc * 128 + 128],
                                  pt[:, 0:128])

    # projb_t[p, mc] = proj_b[mc*128+p]
    st = stg.tile([2, 128], F32, tag="bstage")
    nc.sync.dma_start(out=st[0:2, :],
                      in_=tens["proj_b"][:].rearrange("(m p) -> m p", p=128))
    pt = psum.tile([128, 512], F32, tag="big")
    nc.tensor.transpose(pt[:, 0:2], st[0:2, :], ident[0:2, 0:2])
    projb_t = res.tile([128, 2], F32, tag="projb")
    nc.vector.tensor_copy(projb_t[:, :], pt[:, 0:2])

    # ---- xp = x @ proj_w.T + proj_b   -> bf16 resident ----
    xp = res.tile([128, DKC * ntot], BF16, tag="xp")
    for nt in range(ntot // nch):
        for mc in range(DKC):
            px = psum.tile([128, 512], F32, tag="big")
            for kc in range(DKC):
                nc.tensor.matmul(
                    px[:, 0:nch],
                    projT[:, (kc * 2 + mc) * 128:(kc * 2 + mc) * 128 + 128],
                    xT[:, kc * ntot + nt * nch:kc * ntot + (nt + 1) * nch],
                    start=(kc == 0), stop=(kc == DKC - 1))
            nc.vector.tensor_scalar_add(
                xp[:, mc * ntot + nt * nch:mc * ntot + (nt + 1) * nch],
                px[:, 0:nch], projb_t[:, mc:mc + 1])

    # ---- scan state ----
    c0 = res.tile([128, 64], F32, tag="c0")
    c1 = res.tile([128, 64], F32, tag="c1")
    h0 = res.tile([128, 64], BF16, tag="h0")
    h1 = res.tile([128, 64], BF16, tag="h1")
    for s in (c0, c1):
        nc.vector.memset(s[:, :], 0.0)
    for s in (h0, h1):
        nc.vector.memset(s[:, :], 0.0)

    def l0_tiles():
        return {"A": psum.tile([128, 128], F32, tag="psA", name="psA"),
                "B": psum.tile([128, 64], F32, tag="psB", name="psB"),
                "C": psum.tile([128, 64], F32, tag="psC", name="psC")}

    # one start/stop bracket per psum bank per step: start on the first MM
    # emitted to the tile, stop on the last.
    L0_LAST_OFF = {"A": 112, "B": 48, "C": 48}

    def emit_l0_xside(ps, xps):
        for key, off, mc in L0_SLOTS:
            t_ = ps[key]
            for kc in range(DKC):
                nc.tensor.matmul(
                    t_[:, off:off + 16],
                    w0T[:, (kc * MCH + mc) * 128:(kc * MCH + mc) * 128 + 128],
                    xps[:, kc * 16:(kc + 1) * 16],
                    start=(kc == 0 and off == 0), stop=False)

    def emit_l0_hside(ps):
        for key, off, mc in L0_SLOTS:
            t_ = ps[key]
            for j in range(HKC):
                kc = DKC + j
                nc.tensor.matmul(
                    t_[:, off:off + 16],
                    w0T[:, (kc * MCH + mc) * 128:(kc * MCH + mc) * 128 + 128],
                    h0[:, j * 16:(j + 1) * 16],
                    start=False,
                    stop=(j == HKC - 1 and off == L0_LAST_OFF[key]))

    def act(fn, dst, src):
        nc.scalar.activation(dst, src, fn)

    def chain_l0(ps):
        pa, pb, pc = ps["A"], ps["B"], ps["C"]
        nc.vector.tensor_add(pa[:, :], pa[:, :], biasA[:, :])
        sf = scn.tile([128, 64], F32, tag="sf")
        si = scn.tile([128, 64], F32, tag="si")
        tg = scn.tile([128, 64], F32, tag="tg")
        so = scn.tile([128, 64], F32, tag="so")
        tc_ = scn.tile([128, 64], F32, tag="tc")
        tmp = scn.tile([128, 64], F32, tag="tmp")
        act(AF.Sigmoid, sf[:, :], pa[:, 0:64])
        act(AF.Sigmoid, si[:, :], pa[:, 64:128])
        nc.vector.tensor_mul(c0[:, :], sf[:, :], c0[:, :])
        nc.vector.tensor_add(pb[:, :], pb[:, :], biasB[:, :])
        act(AF.Tanh, tg[:, :], pb[:, :])
        nc.vector.tensor_mul(tmp[:, :], si[:, :], tg[:, :])
        nc.vector.tensor_add(c0[:, :], c0[:, :], tmp[:, :])
        act(AF.Tanh, tc_[:, :], c0[:, :])
        nc.vector.tensor_add(pc[:, :], pc[:, :], biasC[:, :])
        act(AF.Sigmoid, so[:, :], pc[:, :])
        nc.vector.tensor_mul(h0[:, :], so[:, :], tc_[:, :])

    def emit_l1(psl):
        for mc in range(MCH):
            for kc in range(2 * HKC):
                src = h0 if kc < HKC else h1
                j = kc % HKC
                nc.tensor.matmul(
                    psl[:, mc * 16:(mc + 1) * 16],
                    w1T[:, (kc * MCH + mc) * 128:(kc * MCH + mc) * 128 + 128],
                    src[:, j * 16:(j + 1) * 16],
                    start=(kc == 0 and mc == 0),
                    stop=(kc == 2 * HKC - 1 and mc == MCH - 1))

    def chain_l1(psl):
        nc.vector.tensor_add(psl[:, :], psl[:, :], bias1[:, :])
        sf = scn.tile([128, 64], F32, tag="sf1")
        si = scn.tile([128, 64], F32, tag="si1")
        tg = scn.tile([128, 64], F32, tag="tg1")
        so = scn.tile([128, 64], F32, tag="so1")
        tc_ = scn.tile([128, 64], F32, tag="tc1")
        tmp = scn.tile([128, 64], F32, tag="tmp1")
        act(AF.Sigmoid, sf[:, :], psl[:, 64:128])
        act(AF.Sigmoid, si[:, :], psl[:, 0:64])
        nc.vector.tensor_mul(c1[:, :], sf[:, :], c1[:, :])
        act(AF.Tanh, tg[:, :], psl[:, 128:192])
        nc.vector.tensor_mul(tmp[:, :], si[:, :], tg[:, :])
        nc.vector.tensor_add(c1[:, :], c1[:, :], tmp[:, :])
        act(AF.Tanh, tc_[:, :], c1[:, :])
        act(AF.Sigmoid, so[:, :], psl[:, 192:256])
        nc.vector.tensor_mul(h1[:, :], so[:, :], tc_[:, :])

    def fetch_xps(it, u):
        xps = scn.tile([128, 2 * 16], BF16, tag="xps")
        src = xp[:, :].rearrange("p (k n) -> p k n", k=DKC)
        nc.sync.dma_start(
            out=xps[:, :].rearrange("p (k n) -> p k n", k=DKC),
            in_=src[:, :, ds(it * (unroll * 16) + u * 16, 16)])
        return xps

    n_iter = t_steps // unroll
    with tc.For_i(0, n_iter, 1,
                  hint_engines=(mybir.EngineType.PE,)) as it:
        ps_cur = None
        for u in range(unroll):
            if u == 0:
                ps_cur = l0_tiles()
                emit_l0_xside(ps_cur, fetch_xps(it, 0))
            emit_l0_hside(ps_cur)
            chain_l0(ps_cur)
            if u < unroll - 1:
                ps_nxt = l0_tiles()
                emit_l0_xside(ps_nxt, fetch_xps(it, u + 1))
            else:
                ps_nxt = None
            psl = psum.tile([128, 256], F32, tag="big")
            emit_l1(psl)
            chain_l1(psl)
            ps_cur = ps_nxt

    # ---- FC head ----
    ph = psum.tile([128, 512], F32, tag="big")
    for kc in range(HKC):
        nc.tensor.matmul(ph[0:32, 0:16], fc1T[:, kc * 32:(kc + 1) * 32],
                         h1[:, kc * 16:(kc + 1) * 16],
                         start=(kc == 0), stop=(kc == HKC - 1))
    hid = scn.tile([32, 16], BF16, tag="hid")
    nc.scalar.activation(hid[:, :], ph[0:32, 0:16], AF.Relu,
                         bias=fc1b[:, 0:1])
    po = psum.tile([128, 512], F32, tag="big")
    nc.tensor.matmul(po[0:1, 0:16], fc2T[:, 0:1], hid[:, :],
                     start=True, stop=True)
    ob = scn.tile([1, 16], F32, tag="ob")
    nc.vector.tensor_scalar_add(ob[:, :], po[0:1, 0:16], fc2b[0:1, 0:1])
    nc.sync.dma_start(out=out_d[:, :].rearrange("b o -> o b"), in_=ob[:, :])


_NC_CACHE = {}


def _get_nc(t_steps=T, unroll=8):
    key = (t_steps, unroll)
    if key not in _NC_CACHE:
        _NC_CACHE[key] = build_nc(t_steps, unroll)
    return _NC_CACHE[key]


def kernel(**inputs):
    nc = _get_nc()
    arrs = {k: np.ascontiguousarray(np.asarray(v, dtype=np.float32))
            for k, v in inputs.items()}
    in_maps = []
    for c in range(NCORES):
        m = {k: v for k, v in arrs.items() if k != "x"}
        m["x"] = np.ascontiguousarray(arrs["x"][c * BL:(c + 1) * BL])
        in_maps.append(m)
    res = run_bass_kernel_spmd(nc, in_maps, core_ids=list(range(NCORES)))
    return np.concatenate([r["out"] for r in res.results], axis=0)
